# revision 18
# baseline (speedup 1.0000x reference)
"""CktGNN encoder forward on 8 Trainium2 NeuronCores (pure data parallel).

Strategy
--------
- Shard the batch B=4096 across 8 cores (512 graphs/core), replicating the
  small GRU/gate/mapper/MLP parameters.
- Host-side prep is limited to index encodings / layout transforms: one-hot
  node-type/pos, the DAG-mask product A = adj*tri*valid, the df scatter
  (pure data movement), bias-row folding into weight matrices, and a stable
  sort of each core's 512 graphs by vertex count so batch-tiles of 128 get
  static per-tile loop bounds (graphs with few vertices skip most steps).
- On device (per core), batch is processed as 4 tiles of 128 samples
  (batch-major elementwise, feature-major lhsT for matmuls):
    * per vertex v: GRU pre-activations accumulate in PSUM
      (gi[rz]+gh[rz] share one accumulation; the n-gate halves stay split),
    * gate/mapper run once per vertex u (not per (u,v) pair as in the
      reference), producing gated messages G_u,
    * the per-sample weighted aggregation Hin_v += A[b,u,v] * G_u uses
      fused scalar_tensor_tensor MACs on the vector engine,
    * PE transposes keep a feature-major copy of the recurrent state for
      use as matmul lhsT,
    * graph readout Hg = H[b, n_b-1] is a one-hot MAC; heads run
      feature-major once per core and the [112, 512] result is
      un-transposed on the host.
"""
import os
from contextlib import ExitStack

import ml_dtypes
import numpy as np

import concourse.bass as bass
import concourse.mybir as mybir
import concourse.tile as tile
from concourse import bacc
from concourse.bass_utils import run_bass_kernel_spmd
from concourse.masks import make_identity

# Problem shapes (hardcoded per the spec).
B, MAXN = 4096, 10
NVT, MAXPOS = 26, 9
HS, NZ = 301, 56
EMB, FEAT = 16, 8
VS = HS + MAXPOS            # 310
XDIM = NVT + MAXPOS         # 35
DFD = 3 * MAXPOS            # 27
GS = HS + FEAT              # 309
NH = 2 * NZ                 # 112 (mu || logvar)

NCORES = 8
BC = B // NCORES            # 512 per core
P = 128
NT = BC // P                # 4 batch tiles per core

F32 = mybir.dt.float32
BF16 = mybir.dt.bfloat16
NP_BF16 = ml_dtypes.bfloat16
AF = mybir.ActivationFunctionType
ALU = mybir.AluOpType

LAST_RUN_INFO = {}
_BUILD_CACHE = {}


def _pidx(u, v):
    """Flat index of pair (u, v), u < v, in the 45-column A-pair table."""
    return v * (v - 1) // 2 + u


def _host_prep(inputs):
    nt_ = np.asarray(inputs["node_type"]).astype(np.int64)
    pos = np.asarray(inputs["pos"]).astype(np.int64)
    adj = np.asarray(inputs["adj"]).astype(np.float32)
    vcount = np.asarray(inputs["vcount"]).astype(np.int64)
    r = np.asarray(inputs["r"]).astype(np.float32)
    c = np.asarray(inputs["c"]).astype(np.float32)
    gm = np.asarray(inputs["gm"]).astype(np.float32)

    n = np.clip(vcount, 1, MAXN)
    ids = np.arange(MAXN)
    valid = (ids[None, :] < n[:, None]).astype(np.float32)          # [B,10]
    tri = (ids[:, None] < ids[None, :]).astype(np.float32)          # [10,10]
    A = adj * tri[None] * valid[:, :, None] * valid[:, None, :]     # [B,10,10]
    npairs = MAXN * (MAXN - 1) // 2
    A_pairs = np.zeros((B, npairs), np.float32)
    for v in range(1, MAXN):
        for u in range(v):
            A_pairs[:, _pidx(u, v)] = A[:, u, v]

    t_oh = np.eye(NVT, dtype=np.float32)[nt_]                       # [B,10,26]
    p_oh = np.eye(MAXPOS, dtype=np.float32)[pos]                    # [B,10,9]
    Xext = np.concatenate(
        [t_oh, p_oh, np.ones((B, MAXN, 1), np.float32)], axis=-1
    )                                                               # [B,10,36]
    xt = np.ascontiguousarray(Xext.transpose(1, 2, 0)).astype(NP_BF16)  # [10,36,B]
    # pos one-hot plus a ones row (carries the gate bias via the wgm pack).
    p_ohx = np.concatenate([p_oh, np.ones((B, MAXN, 1), np.float32)], axis=-1)
    poht = np.ascontiguousarray(p_ohx.transpose(1, 2, 0)).astype(NP_BF16)

    lastoh = np.eye(MAXN, dtype=np.float32)[n - 1]                  # [B,10]

    # df scatter with later-vertex-overwrites semantics.
    df = np.zeros((B, DFD), np.float32)
    bi = np.arange(B)
    for v in range(MAXN):
        m = valid[:, v] > 0
        for k, val in enumerate((r[:, v], c[:, v], gm[:, v])):
            idx = pos[:, v] * 3 + k
            df[bi[m], idx[m]] = val[m]
    dft = np.concatenate([df.T, np.ones((1, B), np.float32)])       # [28,B]

    # Per-core stratified assignment: stable-sort by n, deal round-robin so
    # every core sees the same n-profile; within a core the 512 samples stay
    # ascending in n, so batch-tiles of 128 have tight vertex-count bounds.
    perm = np.argsort(n, kind="stable")
    core_idx = [perm[k::NCORES] for k in range(NCORES)]

    per_core = []
    for k in range(NCORES):
        ix = core_idx[k]
        per_core.append(
            dict(
                xt=np.ascontiguousarray(xt[:, :, ix]),
                poht=np.ascontiguousarray(poht[:, :, ix]),
                apair=np.ascontiguousarray(A_pairs[ix]),
                lastoh=np.ascontiguousarray(lastoh[ix]),
                dft=np.ascontiguousarray(dft[:, ix]),
            )
        )

    # Static per-tile bounds: max/set over all cores so one program fits all.
    VMAX = []
    HGV = []
    for t in range(NT):
        vmax = 1
        hgset = set()
        for k in range(NCORES):
            ntile = n[core_idx[k]][t * P:(t + 1) * P]
            vmax = max(vmax, int(ntile.max()))
            hgset.update(int(x) for x in np.unique(ntile - 1))
        VMAX.append(vmax)
        HGV.append(sorted(hgset))

    # Weight packs with folded bias rows (ones-row / bias-row trick).
    W_ih = np.asarray(inputs["W_ih"], np.float32)
    W_hh = np.asarray(inputs["W_hh"], np.float32)
    b_ih = np.asarray(inputs["b_ih"], np.float32)
    b_hh = np.asarray(inputs["b_hh"], np.float32)
    Wg = np.asarray(inputs["Wg"], np.float32)
    bg = np.asarray(inputs["bg"], np.float32)
    Wm = np.asarray(inputs["Wm"], np.float32)
    W1 = np.asarray(inputs["W1"], np.float32)
    b1 = np.asarray(inputs["b1"], np.float32)
    W2 = np.asarray(inputs["W2"], np.float32)
    b2 = np.asarray(inputs["b2"], np.float32)
    Wmu = np.asarray(inputs["Wmu"], np.float32)
    bmu = np.asarray(inputs["bmu"], np.float32)
    Wlv = np.asarray(inputs["Wlv"], np.float32)
    blv = np.asarray(inputs["blv"], np.float32)

    # X-side weight pack [36, 1204]: cols 0:903 are W_ih.T with a bias row
    # (r/z cols also absorb b_hh's r/z parts since gi+gh share one PSUM);
    # cols 903:1204 are zeros except the bias row, which carries b_hh's
    # n-part into pg_hn — so the Hin lhsT needs no ones row at all.
    wih = np.zeros((XDIM + 1, 4 * HS), np.float32)
    wih[0:XDIM, 0:3 * HS] = W_ih.T
    wih[XDIM, 0:2 * HS] = b_ih[0:2 * HS] + b_hh[0:2 * HS]
    wih[XDIM, 2 * HS:3 * HS] = b_ih[2 * HS:3 * HS]
    wih[XDIM, 3 * HS:4 * HS] = b_hh[2 * HS:3 * HS]
    whh = np.ascontiguousarray(W_hh.T)                              # [301,903]
    wgm = np.ascontiguousarray(
        np.vstack(
            [
                np.hstack([Wg.T, Wm.T]),                            # [310,602]
                np.hstack([bg, np.zeros(HS, np.float32)])[None],
            ]
        )
    )                                                               # [311,602]
    whead = np.ascontiguousarray(
        np.vstack(
            [
                np.hstack([Wmu.T, Wlv.T]),                          # [309,112]
                np.hstack([bmu, blv])[None],
            ]
        )
    )                                                               # [310,112]
    w1 = np.ascontiguousarray(np.vstack([W1.T, b1[None]]))          # [28,16]
    w2 = np.ascontiguousarray(np.vstack([W2.T, b2[None]]))          # [17,8]
    weights = dict(
        wih=wih.astype(NP_BF16), whh=whh.astype(NP_BF16),
        wgm=wgm.astype(NP_BF16), whead=whead, w1=w1, w2=w2,
    )

    return per_core, weights, core_idx, VMAX, HGV


def _build(VMAX, HGV):
    """Emit the per-core Bass program (same program runs on all 8 cores)."""
    nc = bacc.Bacc(
        "TRN2",
        target_bir_lowering=False,
        debug=False,
        enable_asserts=False,
        num_devices=NCORES,
    )

    d = {}
    d["xt"] = nc.dram_tensor("xt", [MAXN, XDIM + 1, BC], BF16, kind="ExternalInput").ap()
    d["poht"] = nc.dram_tensor("poht", [MAXN, MAXPOS + 1, BC], BF16, kind="ExternalInput").ap()
    d["apair"] = nc.dram_tensor("apair", [BC, 45], F32, kind="ExternalInput").ap()
    d["lastoh"] = nc.dram_tensor("lastoh", [BC, MAXN], F32, kind="ExternalInput").ap()
    d["dft"] = nc.dram_tensor("dft", [DFD + 1, BC], F32, kind="ExternalInput").ap()
    d["wih"] = nc.dram_tensor("wih", [XDIM + 1, 4 * HS], BF16, kind="ExternalInput").ap()
    d["whh"] = nc.dram_tensor("whh", [HS, 3 * HS], BF16, kind="ExternalInput").ap()
    d["wgm"] = nc.dram_tensor("wgm", [VS + 1, 2 * HS], BF16, kind="ExternalInput").ap()
    d["whead"] = nc.dram_tensor("whead", [GS + 1, NH], F32, kind="ExternalInput").ap()
    d["w1"] = nc.dram_tensor("w1", [DFD + 1, EMB], F32, kind="ExternalInput").ap()
    d["w2"] = nc.dram_tensor("w2", [EMB + 1, FEAT], F32, kind="ExternalInput").ap()
    outT = nc.dram_tensor("outT", [NH, BC], F32, kind="ExternalOutput").ap()

    G3 = 3 * HS  # 903
    with tile.TileContext(nc) as tc, ExitStack() as ctx:
        consts = ctx.enter_context(tc.tile_pool(name="consts", bufs=1))
        state = ctx.enter_context(tc.tile_pool(name="state", bufs=1))
        work = ctx.enter_context(tc.tile_pool(name="work", bufs=1))
        psum = ctx.enter_context(tc.tile_pool(name="psum", bufs=1, space="PSUM"))

        ident = consts.tile([P, P], F32, tag="ident")
        make_identity(nc, ident[:])

        wih = consts.tile([XDIM + 1, 4 * HS], BF16, tag="wih")
        nc.sync.dma_start(wih[:], d["wih"][:])
        whh_c = []
        for ci, (r0, r1) in enumerate([(0, 128), (128, 256), (256, 301)]):
            w = consts.tile([r1 - r0, G3], BF16, tag=f"whh{ci}")
            nc.sync.dma_start(w[:], d["whh"][r0:r1, :])
            whh_c.append(w)
        wgm_c = []
        for ci, (r0, r1) in enumerate([(0, 128), (128, 256), (256, 311)]):
            w = consts.tile([r1 - r0, 2 * HS], BF16, tag=f"wgm{ci}")
            nc.sync.dma_start(w[:], d["wgm"][r0:r1, :])
            wgm_c.append(w)
        whead_c = []
        for ci, (r0, r1) in enumerate([(0, 128), (128, 256), (256, 310)]):
            w = consts.tile([r1 - r0, NH], F32, tag=f"whead{ci}")
            nc.sync.dma_start(w[:], d["whead"][r0:r1, :])
            whead_c.append(w)
        w1t = consts.tile([DFD + 1, EMB], F32, tag="w1t")
        nc.sync.dma_start(w1t[:], d["w1"][:])
        w2t = consts.tile([EMB + 1, FEAT], F32, tag="w2t")
        nc.sync.dma_start(w2t[:], d["w2"][:])

        zero = consts.tile([P, HS], F32, tag="zero")
        nc.gpsimd.memset(zero[:], 0.0)

        A_t, last_t, Hg, Hin = [], [], [], []
        for t in range(NT):
            sl = slice(t * P, (t + 1) * P)
            a = state.tile([P, 45], F32, tag=f"A{t}")
            nc.sync.dma_start(a[:], d["apair"][sl, :])
            A_t.append(a)
            lo = state.tile([P, MAXN], F32, tag=f"lo{t}")
            nc.sync.dma_start(lo[:], d["lastoh"][sl, :])
            last_t.append(lo)
            hg = state.tile([P, HS], F32, tag=f"hg{t}")
            nc.gpsimd.memset(hg[:], 0.0)
            Hg.append(hg)
            hv = {}
            for v in range(1, VMAX[t]):
                hh = state.tile([P, HS], F32, tag=f"hin{t}_{v}")
                nc.gpsimd.memset(hh[:], 0.0)
                hv[v] = hh
            Hin.append(hv)

        for v in range(max(VMAX)):
            for t in range(NT):
                if v >= VMAX[t]:
                    continue
                sl = slice(t * P, (t + 1) * P)
                # Feature-major (transposed) copy of Hin_v for use as lhsT.
                hinT = work.tile([P, 384], BF16, tag="hinT", bufs=4)
                if v == 0:
                    hin_v = zero
                    nc.gpsimd.memset(hinT[:], 0.0)
                else:
                    hin_v = Hin[t][v]
                    pt = psum.tile([P, 384], F32, tag="pt", bufs=2)
                    nc.tensor.transpose(pt[:, 0:128], hin_v[:, 0:128], ident[:])
                    nc.tensor.transpose(pt[:, 128:256], hin_v[:, 128:256], ident[:])
                    nc.tensor.transpose(pt[0:45, 256:384], hin_v[:, 256:301], ident[:])
                    nc.vector.tensor_copy(hinT[:, 0:256], pt[:, 0:256])
                    nc.vector.tensor_copy(hinT[0:45, 256:384], pt[0:45, 256:384])

                xv = work.tile([XDIM + 1, P], BF16, tag="xv", bufs=8)
                nc.sync.dma_start(xv[:], d["xt"][v, :, sl])

                # GRU pre-activations. r/z halves of gi and gh accumulate in
                # one PSUM region; the n halves stay separate (hn feeds the
                # r-gated product).
                pg_rz = psum.tile([P, 602], F32, tag="pgrz")
                pg_in = psum.tile([P, HS], F32, tag="pgin")
                pg_hn = psum.tile([P, HS], F32, tag="pghn")
                nc.tensor.matmul(pg_rz[:, 0:512], xv[:], wih[:, 0:512],
                                 start=True, stop=False)
                nc.tensor.matmul(pg_rz[:, 512:602], xv[:], wih[:, 512:602],
                                 start=True, stop=False)
                nc.tensor.matmul(pg_in[:], xv[:], wih[:, 602:903],
                                 start=True, stop=True)
                nc.tensor.matmul(pg_hn[:], xv[:], wih[:, 903:1204],
                                 start=True, stop=False)
                lhs_chunks = [hinT[:, 0:128], hinT[:, 128:256], hinT[0:45, 256:384]]
                for ci, lhs in enumerate(lhs_chunks):
                    last = ci == 2
                    nc.tensor.matmul(pg_rz[:, 0:512], lhs, whh_c[ci][:, 0:512],
                                     start=False, stop=last)
                    nc.tensor.matmul(pg_rz[:, 512:602], lhs, whh_c[ci][:, 512:602],
                                     start=False, stop=last)
                    nc.tensor.matmul(pg_hn[:], lhs, whh_c[ci][:, 602:903],
                                     start=False, stop=last)

                rsig = work.tile([P, HS], F32, tag="rsig", bufs=4)
                nc.scalar.activation(rsig[:], pg_rz[:, 0:HS], AF.Sigmoid)
                zsig = work.tile([P, HS], F32, tag="zsig", bufs=4)
                nc.scalar.activation(zsig[:], pg_rz[:, HS:602], AF.Sigmoid)
                rn = work.tile([P, HS], F32, tag="rn", bufs=4)
                nc.vector.tensor_tensor(rn[:], rsig[:], pg_hn[:], ALU.mult)
                npre = work.tile([P, HS], F32, tag="npre", bufs=4)
                nc.vector.tensor_tensor(npre[:], rn[:], pg_in[:], ALU.add)
                ngt = work.tile([P, HS], F32, tag="ngt", bufs=4)
                nc.scalar.activation(ngt[:], npre[:], AF.Tanh)
                dl = work.tile([P, HS], F32, tag="dl", bufs=4)
                nc.gpsimd.tensor_tensor(dl[:], hin_v[:], ngt[:], ALU.subtract)
                zd = work.tile([P, HS], F32, tag="zd", bufs=4)
                nc.gpsimd.tensor_tensor(zd[:], zsig[:], dl[:], ALU.mult)
                h = work.tile([P, HS], F32, tag="h", bufs=4)
                nc.gpsimd.tensor_tensor(h[:], ngt[:], zd[:], ALU.add)

                # Graph readout: Hg += onehot(n-1)[v] * h_v.
                if v in HGV[t]:
                    nc.vector.scalar_tensor_tensor(
                        Hg[t][:], h[:], last_t[t][:, v:v + 1], Hg[t][:],
                        ALU.mult, ALU.add,
                    )

                # Gated message G_v and its fan-out to later vertices.
                if v <= VMAX[t] - 2:
                    pt2 = psum.tile([P, 384], F32, tag="pt", bufs=2)
                    nc.tensor.transpose(pt2[:, 0:128], h[:, 0:128], ident[:])
                    nc.tensor.transpose(pt2[:, 128:256], h[:, 128:256], ident[:])
                    nc.tensor.transpose(pt2[0:45, 256:384], h[:, 256:301], ident[:])
                    hcatT = work.tile([P, 384], BF16, tag="hcatT", bufs=4)
                    nc.vector.tensor_copy(hcatT[:, 0:256], pt2[:, 0:256])
                    nc.vector.tensor_copy(hcatT[0:45, 256:384], pt2[0:45, 256:384])
                    nc.sync.dma_start(hcatT[45:55, 256:384], d["poht"][v, :, sl])

                    pgm = psum.tile([P, 602], F32, tag="pgm")
                    hc_chunks = [hcatT[:, 0:128], hcatT[:, 128:256], hcatT[0:55, 256:384]]
                    for ci, lhs in enumerate(hc_chunks):
                        nc.tensor.matmul(pgm[:, 0:512], lhs, wgm_c[ci][:, 0:512],
                                         start=(ci == 0), stop=(ci == 2))
                        nc.tensor.matmul(pgm[:, 512:602], lhs, wgm_c[ci][:, 512:602],
                                         start=(ci == 0), stop=(ci == 2))
                    sg = work.tile([P, HS], F32, tag="sg", bufs=4)
                    nc.scalar.activation(sg[:], pgm[:, 0:HS], AF.Sigmoid)
                    G = work.tile([P, HS], F32, tag="G", bufs=4)
                    nc.vector.tensor_tensor(G[:], sg[:], pgm[:, HS:602], ALU.mult)
                    for w in range(v + 1, VMAX[t]):
                        p = _pidx(v, w)
                        nc.vector.scalar_tensor_tensor(
                            Hin[t][w][:], G[:], A_t[t][:, p:p + 1], Hin[t][w][:],
                            ALU.mult, ALU.add,
                        )

        # Readout heads, feature-major over the whole core (N = 512).
        rhs1 = state.tile([P, BC], F32, tag="hr1")
        rhs2 = state.tile([P, BC], F32, tag="hr2")
        rhs3 = state.tile([54, BC], F32, tag="hr3")
        for t in range(NT):
            cs = slice(t * P, (t + 1) * P)
            pt = psum.tile([P, 384], F32, tag="pt", bufs=2)
            nc.tensor.transpose(pt[:, 0:128], Hg[t][:, 0:128], ident[:])
            nc.tensor.transpose(pt[:, 128:256], Hg[t][:, 128:256], ident[:])
            nc.tensor.transpose(pt[0:45, 256:384], Hg[t][:, 256:301], ident[:])
            nc.vector.tensor_copy(rhs1[:, cs], pt[:, 0:128])
            nc.vector.tensor_copy(rhs2[:, cs], pt[:, 128:256])
            nc.vector.tensor_copy(rhs3[0:45, cs], pt[0:45, 256:384])

        dftile = work.tile([DFD + 1, BC], F32, tag="dftile")
        nc.sync.dma_start(dftile[:], d["dft"][:])
        p1 = psum.tile([EMB, BC], F32, tag="pt", bufs=2)
        nc.tensor.matmul(p1[:], w1t[:], dftile[:], start=True, stop=True)
        e1 = work.tile([EMB + 1, BC], F32, tag="e1")
        nc.scalar.activation(e1[0:EMB, :], p1[:], AF.Relu)
        # Ones rows at non-32-aligned partitions must be written by DMA
        # (compute engines require 32-aligned partition bases). dft's last
        # row is already all-ones.
        nc.sync.dma_start(e1[EMB:EMB + 1, :], d["dft"][DFD:DFD + 1, :])
        p2 = psum.tile([FEAT, BC], F32, tag="pghn")
        nc.tensor.matmul(p2[:], w2t[:], e1[:], start=True, stop=True)
        hd8 = work.tile([FEAT, BC], F32, tag="hd8")
        nc.vector.tensor_copy(hd8[:], p2[:])
        nc.sync.dma_start(rhs3[45:53, :], hd8[:])
        nc.sync.dma_start(rhs3[53:54, :], d["dft"][DFD:DFD + 1, :])

        po = psum.tile([NH, BC], F32, tag="pgrz")
        head_rhs = [rhs1, rhs2, rhs3]
        for ci in range(3):
            nc.tensor.matmul(po[:], whead_c[ci][:], head_rhs[ci][:],
                             start=(ci == 0), stop=(ci == 2))
        osb = work.tile([NH, BC], F32, tag="osb")
        nc.vector.tensor_copy(osb[:], po[:])
        nc.sync.dma_start(outT[:], osb[:])

    nc.compile()
    return nc


def kernel(**inputs):
    per_core, weights, core_idx, VMAX, HGV = _host_prep(inputs)

    key = (tuple(VMAX), tuple(tuple(h) for h in HGV))
    if key not in _BUILD_CACHE:
        _BUILD_CACHE[key] = _build(VMAX, HGV)
    nc = _BUILD_CACHE[key]

    in_maps = []
    for k in range(NCORES):
        m = dict(per_core[k])
        m.update(weights)
        in_maps.append(m)

    trace = bool(int(os.environ.get("KERNEL_TRACE", "0")))
    res = run_bass_kernel_spmd(nc, in_maps, list(range(NCORES)), trace=trace)

    LAST_RUN_INFO.clear()
    LAST_RUN_INFO["exec_time_ns"] = res.exec_time_ns
    LAST_RUN_INFO["VMAX"] = VMAX

    mu = np.zeros((B, NZ), np.float32)
    lv = np.zeros((B, NZ), np.float32)
    for k in range(NCORES):
        o = res.results[k]["outT"]
        mu[core_idx[k]] = o[0:NZ].T
        lv[core_idx[k]] = o[NZ:NH].T
    return mu, lv
